# revision 49
# baseline (speedup 1.0000x reference)
"""Distributed causal self-attention for TRN2 (8 NeuronCores).

Problem: B=4, T=2048, C=1024, H=16 heads, D=64.
  qkv = x @ W_qkv + b_qkv ; causal softmax attention ; y @ W_proj + b_proj

Sharding (8 cores): core c -> batch b = c//2, head-group g = c%2
(heads 8g..8g+7).  Each core computes, for its (b, g):
  Q^T/K^T (hd, T) and V (T, hd) for its 8 heads (hd = 512),
  flash-style causal attention in S^T = K @ Q^T layout (s on partitions,
  head pairs row/col-packed on the PE array),
  partial out^T = (Y @ W_proj[rows g])^T  (1024, 2048).
Host unshard: out[b] = (part[2b] + part[2b+1]).T  (b_proj added on-device
by the g==0 core only).

Host-side shard prep casts matmul operands to bf16 and pre-transposes x
(to x^T, the layout every projection consumes); PSUM accumulates fp32.
Softmax runs without max-subtraction (scores ~ N(0,1) here; exp is safe
in fp32), the padding mask is folded into the exp bias, and the causal
mask is a (128,128) triangular multiply on diagonal tiles.  The softmax
denominator comes free from an all-ones 65th column appended to V;
normalization happens on the (64, 512) attention output tiles via a
DRAM-bounced partition broadcast of 1/l (PE outer-product broadcast on
the tail chunk, where the PE is otherwise idle).
"""

from contextlib import ExitStack

import numpy as np

# ---------------- constants (hardcoded per problem spec) ----------------
B, T, C, H, D = 4, 2048, 1024, 16, 64
HD = 512          # heads-per-core * D = 8 * 64
NK = C // 128     # 8 contraction tiles over C
NM = HD // 128    # 4 tiles over the per-core head dim (also = head pairs)
NT = T // 128     # 16 s/T blocks
NCH = T // 512    # 4 q-chunks
SCALE = 1.0 / np.sqrt(D)  # 0.125
NEG = -30.0       # "minus infinity" for the padding mask bias


def build_nc():
    import concourse.bass as bass
    import concourse.mybir as mybir
    import concourse.tile as tile
    from concourse.bacc import Bacc

    f32 = mybir.dt.float32
    bf16 = mybir.dt.bfloat16
    Exp = mybir.ActivationFunctionType.Exp
    ADD = mybir.AluOpType.add

    nc = Bacc()

    xt_d = nc.dram_tensor("xt", (C, T), bf16, kind="ExternalInput")
    wqkv_d = nc.dram_tensor("wqkv", (C, 3 * HD), bf16, kind="ExternalInput")
    bq_d = nc.dram_tensor("bq", (HD,), f32, kind="ExternalInput")
    bk_d = nc.dram_tensor("bk", (HD,), f32, kind="ExternalInput")
    bv_d = nc.dram_tensor("bv520", (8 * 65,), bf16, kind="ExternalInput")
    wp_d = nc.dram_tensor("wproj", (HD, C), bf16, kind="ExternalInput")
    bp_d = nc.dram_tensor("bproj", (C,), f32, kind="ExternalInput")
    mb_d = nc.dram_tensor("maskbias", (128, NT), f32, kind="ExternalInput")
    tri_d = nc.dram_tensor("tri", (128, 128), bf16, kind="ExternalInput")
    out_d = nc.dram_tensor("out", (C, T), f32, kind="ExternalOutput")

    ts = bass.ts

    with ExitStack() as ctx:
        tc = ctx.enter_context(tile.TileContext(nc))
        persist = ctx.enter_context(tc.tile_pool(name="persist", bufs=1))
        small = ctx.enter_context(tc.tile_pool(name="small", bufs=1))
        ppool = ctx.enter_context(tc.tile_pool(name="ppool", bufs=6))
        rpool = ctx.enter_context(tc.tile_pool(name="rpool", bufs=2))
        opool = ctx.enter_context(tc.tile_pool(name="opool", bufs=2))
        dram = ctx.enter_context(tc.tile_pool(name="dram", bufs=1, space="DRAM"))
        psum1 = ctx.enter_context(tc.tile_pool(name="psum1", bufs=2, space="PSUM"))
        psS = ctx.enter_context(tc.tile_pool(name="psS", bufs=2, space="PSUM"))
        psY = ctx.enter_context(tc.tile_pool(name="psY", bufs=2, space="PSUM"))

        # ---------------- persistent SBUF tensors ----------------
        XT = persist.tile([128, NK, T], bf16, tag="xt")        # x^T  (C, T)
        WQKV = persist.tile([128, NK, 3 * HD], bf16, tag="wqkv")
        WP = persist.tile([128, NM, C], bf16, tag="wp")
        QT = persist.tile([128, NM, T], bf16, tag="qt")        # Q^T (hd, T)
        KT = persist.tile([128, NM, T], bf16, tag="kt")
        VS = persist.tile([128, NT, 8 * 65], bf16, tag="vs")   # V+ones per s-block
        YT = persist.tile([128, NM, T], bf16, tag="yt")        # normalized Y^T

        rD = dram.tile([2 * NM * NCH, 512], bf16, tag="rd")    # 1/l bounce

        # small constants
        bq_sb = small.tile([128, NM], f32, tag="bq")
        bk_sb = small.tile([128, NM], f32, tag="bk")
        bp_sb = small.tile([128, C // 128], f32, tag="bp")
        mb_sb = small.tile([128, NT], f32, tag="mb")
        tri_b = small.tile([128, 128], bf16, tag="trib")
        bvb = small.tile([128, 8 * 65], bf16, tag="bvb")
        ones_r = small.tile([1, 128], bf16, tag="ones_r")
        nc.vector.memset(ones_r, 1.0)

        nc.gpsimd.dma_start(out=bq_sb, in_=bq_d.rearrange("(m p) -> p m", p=128))
        nc.gpsimd.dma_start(out=bk_sb, in_=bk_d.rearrange("(m p) -> p m", p=128))
        nc.gpsimd.dma_start(out=bp_sb, in_=bp_d.rearrange("(m p) -> p m", p=128))
        nc.gpsimd.dma_start(out=mb_sb, in_=mb_d[:, :])
        nc.gpsimd.dma_start(out=tri_b, in_=tri_d[:, :])
        # broadcast the (520,) v-bias row to 128 partitions via a step-0 AP
        bvb_bcast = bass.AP(tensor=bv_d, offset=0, ap=[[0, 128], [1, 8 * 65]])
        nc.gpsimd.dma_start(out=bvb, in_=bvb_bcast)

        # ---------------- input loads (bf16, no staging) ----------------
        hwdge = [nc.sync, nc.scalar]
        for k in range(NK):
            hwdge[k % 2].dma_start(out=XT[:, k, :], in_=xt_d[ts(k, 128), :])
        for k in range(NK):
            nc.gpsimd.dma_start(out=WQKV[:, k, :], in_=wqkv_d[ts(k, 128), :])

        Ident = mybir.ActivationFunctionType.Identity

        def emit_qk(m, ch):
            # early chunks: drain psum->SBUF on the ACT engine (idle during
            # the QKV-heavy window) so the DVE queue never gates the first
            # S-matmuls' K^T/Q^T readiness
            on_act = ch < 2

            psq = psum1.tile([128, 512], f32, tag="p1")
            for k in range(NK):
                nc.tensor.matmul(
                    psq, WQKV[:, k, ts(m, 128)], XT[:, k, ts(ch, 512)],
                    start=(k == 0), stop=(k == NK - 1),
                )
            if on_act:
                nc.scalar.activation(
                    out=QT[:, m, ts(ch, 512)], in_=psq, func=Ident,
                    bias=bq_sb[:, m : m + 1],
                )
            else:
                nc.vector.tensor_scalar(
                    out=QT[:, m, ts(ch, 512)], in0=psq,
                    scalar1=bq_sb[:, m : m + 1], scalar2=None, op0=ADD,
                )
            psk = psum1.tile([128, 512], f32, tag="p1")
            for k in range(NK):
                nc.tensor.matmul(
                    psk, WQKV[:, k, HD + 128 * m : HD + 128 * (m + 1)],
                    XT[:, k, ts(ch, 512)],
                    start=(k == 0), stop=(k == NK - 1),
                )
            if on_act:
                nc.scalar.activation(
                    out=KT[:, m, ts(ch, 512)], in_=psk, func=Ident,
                    bias=bk_sb[:, m : m + 1],
                )
            else:
                nc.vector.tensor_scalar(
                    out=KT[:, m, ts(ch, 512)], in0=psk,
                    scalar1=bk_sb[:, m : m + 1], scalar2=None, op0=ADD,
                )

        def emit_v(t):
            psv = psum1.tile([128, 512], f32, tag="p1")
            for k in range(NK):
                nc.tensor.matmul(
                    psv, XT[:, k, ts(t, 128)], WQKV[:, k, 2 * HD : 3 * HD],
                    start=(k == 0), stop=(k == NK - 1),
                )
            v3 = VS[:, t, :].rearrange("p (h j) -> p h j", j=65)
            nc.vector.memset(v3[:, :, 64:65], 1.0)
            if t < 8:  # early chunks: drain on the then-idle ACT engine
                nc.scalar.activation(
                    out=v3[:, :, 0:64],
                    in_=psv.rearrange("p (h j) -> p h j", j=64), func=Ident,
                )
            else:
                nc.vector.tensor_copy(
                    out=v3[:, :, 0:64], in_=psv.rearrange("p (h j) -> p h j", j=64)
                )
            nc.vector.tensor_add(out=VS[:, t, :], in0=VS[:, t, :], in1=bvb)

        def emit_attention(pr, ch):
            ypA = psY.tile([65, 512], f32, tag="yp")
            ypB = psY.tile([65, 512], f32, tag="yp")
            nsb = 4 * ch + 4
            for i in range(nsb):
                off = max(0, 128 * i - 512 * ch)
                ncol = 512 - off
                qs = slice(512 * ch + off, 512 * (ch + 1))
                sps = psS.tile([128, 2, 512], f32, tag="s")
                nc.tensor.matmul(
                    sps[:, 0, 0:ncol], KT[0:64, pr, ts(i, 128)], QT[0:64, pr, qs],
                    start=True, stop=True, tile_position=(0, 0),
                )
                nc.tensor.matmul(
                    sps[:, 1, 0:ncol], KT[64:128, pr, ts(i, 128)], QT[64:128, pr, qs],
                    start=True, stop=True, tile_position=(64, 0),
                )
                pt = ppool.tile([128, 2, 512], bf16, tag="pt")
                nc.scalar.activation(
                    out=pt[:, :, 0:ncol], in_=sps[:, :, 0:ncol],
                    func=Exp, scale=SCALE, bias=mb_sb[:, i : i + 1],
                )
                if 128 * i >= 512 * ch:  # diagonal block: causal mask
                    nc.vector.tensor_mul(
                        out=pt[:, 0, 0:128], in0=pt[:, 0, 0:128], in1=tri_b
                    )
                    nc.vector.tensor_mul(
                        out=pt[:, 1, 0:128], in0=pt[:, 1, 0:128], in1=tri_b
                    )
                v3 = VS[:, i, :].rearrange("p (h j) -> p h j", j=65)
                nc.tensor.matmul(
                    ypA[:, off : off + ncol], v3[:, 2 * pr, :], pt[:, 0, 0:ncol],
                    start=(i == 0), stop=(i == nsb - 1),
                )
                nc.tensor.matmul(
                    ypB[:, off : off + ncol], v3[:, 2 * pr + 1, :], pt[:, 1, 0:ncol],
                    start=(i == 0), stop=(i == nsb - 1),
                )
            # normalize: row 64 of yp holds the softmax denominator.
            # Reciprocal + copy free the PSUM banks fast; the broadcast and
            # multiply run off the critical path on SBUF copies.
            if ch == NCH - 1:
                # tail chunk: broadcast 1/l on the (now idle) PE via an
                # outer product instead of the DRAM bounce — shorter chain
                yraw3 = rpool.tile([128, 512], f32, tag="yraw3")
                for hh, yp in ((0, ypA), (1, ypB)):
                    r1 = rpool.tile([1, 512], bf16, tag=f"r1{hh}")
                    with nc.allow_low_precision(reason="softmax 1/l in bf16"):
                        nc.vector.reciprocal(out=r1, in_=yp[64:65, :])
                    nc.vector.tensor_copy(
                        out=yraw3[64 * hh : 64 * hh + 64, :], in_=yp[0:64, :]
                    )
                    rb_ps = psum1.tile([128, 512], f32, tag="p1")
                    nc.tensor.matmul(rb_ps, ones_r, r1, start=True, stop=True)
                    nc.vector.tensor_mul(
                        out=YT[64 * hh : 64 * hh + 64, pr, ts(ch, 512)],
                        in0=yraw3[64 * hh : 64 * hh + 64, :],
                        in1=rb_ps[64 * hh : 64 * hh + 64, :],
                    )
                return
            yraw = rpool.tile([128, 512], bf16, tag="yraw")
            idx = pr * NCH + ch
            rDi = rD[2 * idx : 2 * idx + 2, :]
            for hh, yp in ((0, ypA), (1, ypB)):
                r1 = rpool.tile([1, 512], bf16, tag=f"r1{hh}")
                with nc.allow_low_precision(reason="softmax 1/l in bf16"):
                    nc.vector.reciprocal(out=r1, in_=yp[64:65, :])
                nc.sync.dma_start(out=rDi[hh : hh + 1, :], in_=r1)
                nc.vector.tensor_copy(
                    out=yraw[64 * hh : 64 * hh + 64, :], in_=yp[0:64, :]
                )
            # broadcast both rows to all 128 partitions via a DRAM bounce
            # with a step-0 partition AP (SBUF APs can't have zero p-step);
            # full-height so each TT reads in0/in1 at the same base partition
            rb = rpool.tile([128, 2, 512], bf16, tag="rb")
            bc = bass.AP(
                tensor=rDi.tensor, offset=rDi.offset,
                ap=[[0, 128], [512, 2], [1, 512]],
            )
            nc.sync.dma_start(out=rb, in_=bc)
            for hh in (0, 1):
                nc.vector.tensor_mul(
                    out=YT[64 * hh : 64 * hh + 64, pr, ts(ch, 512)],
                    in0=yraw[64 * hh : 64 * hh + 64, :],
                    in1=rb[64 * hh : 64 * hh + 64, hh, :],
                )

        def emit_proj(ch):
            # out^T[:, chunk ch] only needs Y[:, :, ch] — run as soon as all
            # pairs' attention for chunk ch is done.  Contract k starting
            # from the last-finishing pair so a chain never parks mid-way on
            # a psum slot waiting for pair 3's Y.
            korder = [NM - 1] + list(range(NM - 1))
            for m in range(C // 128):
                pp = psum1.tile([128, 512], f32, tag="p1")
                for j, k in enumerate(korder):
                    nc.tensor.matmul(
                        pp, WP[:, k, ts(m, 128)], YT[:, k, ts(ch, 512)],
                        start=(j == 0), stop=(j == NM - 1),
                    )
                osb = opool.tile([128, 512], f32, tag="o")
                nc.vector.tensor_scalar(
                    out=osb, in0=pp,
                    scalar1=bp_sb[:, m : m + 1], scalar2=None, op0=ADD,
                )
                eng = nc.sync if ch == NCH - 1 else nc.gpsimd
                eng.dma_start(out=out_d[ts(m, 128), ts(ch, 512)], in_=osb)

        # ------- QKV, attention and proj interleaved per chunk -------
        # attention(pr, ch) needs Q/K chunks 0..ch and V s-blocks 0..4ch+3,
        # all available once QKV chunk ch is emitted.  proj(ch-1) is slotted
        # after QK(ch) so its psum1 chains never interleave with QKV's, and
        # its PE work fills the ACT-bound attention windows.
        for ch in range(NCH):
            if ch == 0:
                # chunk 0: V first, then attention right behind each pair's
                # QK so the exps (ACT) start as early as possible
                for t in range(4):
                    emit_v(t)
                for pr in range(NM):
                    emit_qk(pr, 0)
                    emit_attention(pr, 0)
                continue
            for m in range(NM):
                emit_qk(m, ch)
            for t in range(4 * ch, 4 * ch + 4):
                emit_v(t)
            if ch == 1:  # W_proj loads, needed just before proj(0)
                for k in range(NM):
                    nc.gpsimd.dma_start(out=WP[:, k, :], in_=wp_d[ts(k, 128), :])
            emit_proj(ch - 1)
            for pr in range(NM):
                emit_attention(pr, ch)
        emit_proj(NCH - 1)

    if not nc.is_finalized():
        nc.finalize()
    return nc


def make_in_maps(x, attn_mask, W_qkv, b_qkv, W_proj, b_proj):
    """Shard full inputs into 8 per-core input maps (bf16 matmul operands)."""
    import ml_dtypes

    bf16 = ml_dtypes.bfloat16
    x = np.asarray(x, dtype=np.float32)
    attn_mask = np.asarray(attn_mask)
    W_qkv = np.asarray(W_qkv, dtype=np.float32)
    b_qkv = np.asarray(b_qkv, dtype=np.float32)
    W_proj = np.asarray(W_proj, dtype=np.float32)
    b_proj = np.asarray(b_proj, dtype=np.float32)

    in_maps = []
    for c in range(8):
        b, g = c // 2, c % 2
        s = 512 * g
        wq = W_qkv[:, s : s + 512]
        wk = W_qkv[:, C + s : C + s + 512]
        wv = W_qkv[:, 2 * C + s : 2 * C + s + 512]
        bv = b_qkv[2 * C + s : 2 * C + s + 512]
        bv520 = np.zeros(8 * 65, dtype=np.float32)
        bv520.reshape(8, 65)[:, :64] = bv.reshape(8, 64)
        mb = np.where(
            attn_mask[b].reshape(NT, 128).T.astype(np.int64) != 0, 0.0, NEG
        ).astype(np.float32)
        in_maps.append({
            "xt": np.ascontiguousarray(x[b].T).astype(bf16),
            "wqkv": np.ascontiguousarray(
                np.concatenate([wq, wk, wv], axis=1)
            ).astype(bf16),
            "bq": np.ascontiguousarray(b_qkv[s : s + 512]),
            "bk": np.ascontiguousarray(b_qkv[C + s : C + s + 512]),
            "bv520": bv520.astype(bf16),
            "wproj": np.ascontiguousarray(W_proj[s : s + 512, :]).astype(bf16),
            "bproj": (b_proj if g == 0 else np.zeros(C, dtype=np.float32)).copy(),
            "maskbias": np.ascontiguousarray(mb),
            "tri": np.triu(np.ones((128, 128), dtype=np.float32)).astype(bf16),
        })
    return in_maps


def unshard(results):
    """results: list of 8 dicts with 'out' (C, T) partial transposed outputs."""
    outs = []
    for b in range(B):
        part = results[2 * b]["out"] + results[2 * b + 1]["out"]
        outs.append(part.T)
    return np.ascontiguousarray(np.stack(outs)).astype(np.float32)


def kernel(x, attn_mask, W_qkv, b_qkv, W_proj, b_proj):
    from concourse.bass_utils import run_bass_kernel_spmd

    nc = build_nc()
    in_maps = make_in_maps(x, attn_mask, W_qkv, b_qkv, W_proj, b_proj)
    res = run_bass_kernel_spmd(nc, in_maps, core_ids=list(range(8)))
    kernel.last_results = res
    return unshard([r for r in res.results])


# revision 50
# speedup vs baseline: 1.0058x; 1.0058x over previous
"""Distributed causal self-attention for TRN2 (8 NeuronCores).

Problem: B=4, T=2048, C=1024, H=16 heads, D=64.
  qkv = x @ W_qkv + b_qkv ; causal softmax attention ; y @ W_proj + b_proj

Sharding (8 cores): core c -> batch b = c//2, head-group g = c%2
(heads 8g..8g+7).  Each core computes, for its (b, g):
  Q^T/K^T (hd, T) and V (T, hd) for its 8 heads (hd = 512),
  flash-style causal attention in S^T = K @ Q^T layout (s on partitions,
  head pairs row/col-packed on the PE array),
  partial out^T = (Y @ W_proj[rows g])^T  (1024, 2048).
Host unshard: out[b] = (part[2b] + part[2b+1]).T  (b_proj added on-device
by the g==0 core only).

Host-side shard prep casts matmul operands to bf16 and pre-transposes x
(to x^T, the layout every projection consumes); PSUM accumulates fp32.
Softmax runs without max-subtraction (scores ~ N(0,1) here; exp is safe
in fp32), the padding mask is folded into the exp bias, and the causal
mask is a (128,128) triangular multiply on diagonal tiles.  The softmax
denominator comes free from an all-ones 65th column appended to V;
normalization happens on the (64, 512) attention output tiles via a
DRAM-bounced partition broadcast of 1/l (PE outer-product broadcast on
the tail chunk, where the PE is otherwise idle).
"""

from contextlib import ExitStack

import numpy as np

# ---------------- constants (hardcoded per problem spec) ----------------
B, T, C, H, D = 4, 2048, 1024, 16, 64
HD = 512          # heads-per-core * D = 8 * 64
NK = C // 128     # 8 contraction tiles over C
NM = HD // 128    # 4 tiles over the per-core head dim (also = head pairs)
NT = T // 128     # 16 s/T blocks
NCH = T // 512    # 4 q-chunks
SCALE = 1.0 / np.sqrt(D)  # 0.125
NEG = -30.0       # "minus infinity" for the padding mask bias


def build_nc():
    import concourse.bass as bass
    import concourse.mybir as mybir
    import concourse.tile as tile
    from concourse.bacc import Bacc

    f32 = mybir.dt.float32
    bf16 = mybir.dt.bfloat16
    Exp = mybir.ActivationFunctionType.Exp
    ADD = mybir.AluOpType.add

    nc = Bacc()

    xt_d = nc.dram_tensor("xt", (C, T), bf16, kind="ExternalInput")
    wqkv_d = nc.dram_tensor("wqkv", (C, 3 * HD), bf16, kind="ExternalInput")
    bq_d = nc.dram_tensor("bq", (HD,), f32, kind="ExternalInput")
    bk_d = nc.dram_tensor("bk", (HD,), f32, kind="ExternalInput")
    bv_d = nc.dram_tensor("bv520", (8 * 65,), bf16, kind="ExternalInput")
    wp_d = nc.dram_tensor("wproj", (HD, C), bf16, kind="ExternalInput")
    bp_d = nc.dram_tensor("bproj", (C,), f32, kind="ExternalInput")
    mb_d = nc.dram_tensor("maskbias", (128, NT), f32, kind="ExternalInput")
    tri_d = nc.dram_tensor("tri", (128, 128), bf16, kind="ExternalInput")
    out_d = nc.dram_tensor("out", (C, T), f32, kind="ExternalOutput")

    ts = bass.ts

    with ExitStack() as ctx:
        tc = ctx.enter_context(tile.TileContext(nc))
        persist = ctx.enter_context(tc.tile_pool(name="persist", bufs=1))
        small = ctx.enter_context(tc.tile_pool(name="small", bufs=1))
        ppool = ctx.enter_context(tc.tile_pool(name="ppool", bufs=6))
        rpool = ctx.enter_context(tc.tile_pool(name="rpool", bufs=3))
        opool = ctx.enter_context(tc.tile_pool(name="opool", bufs=3))
        dram = ctx.enter_context(tc.tile_pool(name="dram", bufs=1, space="DRAM"))
        psum1 = ctx.enter_context(tc.tile_pool(name="psum1", bufs=2, space="PSUM"))
        psS = ctx.enter_context(tc.tile_pool(name="psS", bufs=2, space="PSUM"))
        psY = ctx.enter_context(tc.tile_pool(name="psY", bufs=2, space="PSUM"))

        # ---------------- persistent SBUF tensors ----------------
        XT = persist.tile([128, NK, T], bf16, tag="xt")        # x^T  (C, T)
        WQKV = persist.tile([128, NK, 3 * HD], bf16, tag="wqkv")
        WP = persist.tile([128, NM, C], bf16, tag="wp")
        QT = persist.tile([128, NM, T], bf16, tag="qt")        # Q^T (hd, T)
        KT = persist.tile([128, NM, T], bf16, tag="kt")
        VS = persist.tile([128, NT, 8 * 65], bf16, tag="vs")   # V+ones per s-block
        YT = persist.tile([128, NM, T], bf16, tag="yt")        # normalized Y^T

        rD = dram.tile([2 * NM * NCH, 512], bf16, tag="rd")    # 1/l bounce

        # small constants
        bq_sb = small.tile([128, NM], f32, tag="bq")
        bk_sb = small.tile([128, NM], f32, tag="bk")
        bp_sb = small.tile([128, C // 128], f32, tag="bp")
        mb_sb = small.tile([128, NT], f32, tag="mb")
        tri_b = small.tile([128, 128], bf16, tag="trib")
        bvb = small.tile([128, 8 * 65], bf16, tag="bvb")
        ones_r = small.tile([1, 128], bf16, tag="ones_r")
        nc.vector.memset(ones_r, 1.0)

        nc.gpsimd.dma_start(out=bq_sb, in_=bq_d.rearrange("(m p) -> p m", p=128))
        nc.gpsimd.dma_start(out=bk_sb, in_=bk_d.rearrange("(m p) -> p m", p=128))
        nc.gpsimd.dma_start(out=bp_sb, in_=bp_d.rearrange("(m p) -> p m", p=128))
        nc.gpsimd.dma_start(out=mb_sb, in_=mb_d[:, :])
        nc.gpsimd.dma_start(out=tri_b, in_=tri_d[:, :])
        # broadcast the (520,) v-bias row to 128 partitions via a step-0 AP
        bvb_bcast = bass.AP(tensor=bv_d, offset=0, ap=[[0, 128], [1, 8 * 65]])
        nc.gpsimd.dma_start(out=bvb, in_=bvb_bcast)

        # ---------------- input loads (bf16, no staging) ----------------
        hwdge = [nc.sync, nc.scalar]
        for k in range(NK):
            hwdge[k % 2].dma_start(out=XT[:, k, :], in_=xt_d[ts(k, 128), :])
        for k in range(NK):
            nc.gpsimd.dma_start(out=WQKV[:, k, :], in_=wqkv_d[ts(k, 128), :])

        Ident = mybir.ActivationFunctionType.Identity

        def emit_qk(m, ch):
            # early chunks: drain psum->SBUF on the ACT engine (idle during
            # the QKV-heavy window) so the DVE queue never gates the first
            # S-matmuls' K^T/Q^T readiness
            on_act = ch < 2

            psq = psum1.tile([128, 512], f32, tag="p1")
            for k in range(NK):
                nc.tensor.matmul(
                    psq, WQKV[:, k, ts(m, 128)], XT[:, k, ts(ch, 512)],
                    start=(k == 0), stop=(k == NK - 1),
                )
            if on_act:
                nc.scalar.activation(
                    out=QT[:, m, ts(ch, 512)], in_=psq, func=Ident,
                    bias=bq_sb[:, m : m + 1],
                )
            else:
                nc.vector.tensor_scalar(
                    out=QT[:, m, ts(ch, 512)], in0=psq,
                    scalar1=bq_sb[:, m : m + 1], scalar2=None, op0=ADD,
                )
            psk = psum1.tile([128, 512], f32, tag="p1")
            for k in range(NK):
                nc.tensor.matmul(
                    psk, WQKV[:, k, HD + 128 * m : HD + 128 * (m + 1)],
                    XT[:, k, ts(ch, 512)],
                    start=(k == 0), stop=(k == NK - 1),
                )
            if on_act:
                nc.scalar.activation(
                    out=KT[:, m, ts(ch, 512)], in_=psk, func=Ident,
                    bias=bk_sb[:, m : m + 1],
                )
            else:
                nc.vector.tensor_scalar(
                    out=KT[:, m, ts(ch, 512)], in0=psk,
                    scalar1=bk_sb[:, m : m + 1], scalar2=None, op0=ADD,
                )

        def emit_v(t):
            psv = psum1.tile([128, 512], f32, tag="p1")
            for k in range(NK):
                nc.tensor.matmul(
                    psv, XT[:, k, ts(t, 128)], WQKV[:, k, 2 * HD : 3 * HD],
                    start=(k == 0), stop=(k == NK - 1),
                )
            v3 = VS[:, t, :].rearrange("p (h j) -> p h j", j=65)
            nc.vector.memset(v3[:, :, 64:65], 1.0)
            if t < 8:  # early chunks: drain on the then-idle ACT engine
                nc.scalar.activation(
                    out=v3[:, :, 0:64],
                    in_=psv.rearrange("p (h j) -> p h j", j=64), func=Ident,
                )
            else:
                nc.vector.tensor_copy(
                    out=v3[:, :, 0:64], in_=psv.rearrange("p (h j) -> p h j", j=64)
                )
            nc.vector.tensor_add(out=VS[:, t, :], in0=VS[:, t, :], in1=bvb)

        def emit_attention(pr, ch):
            ypA = psY.tile([65, 512], f32, tag="yp")
            ypB = psY.tile([65, 512], f32, tag="yp")
            nsb = 4 * ch + 4
            for i in range(nsb):
                off = max(0, 128 * i - 512 * ch)
                ncol = 512 - off
                qs = slice(512 * ch + off, 512 * (ch + 1))
                sps = psS.tile([128, 2, 512], f32, tag="s")
                nc.tensor.matmul(
                    sps[:, 0, 0:ncol], KT[0:64, pr, ts(i, 128)], QT[0:64, pr, qs],
                    start=True, stop=True, tile_position=(0, 0),
                )
                nc.tensor.matmul(
                    sps[:, 1, 0:ncol], KT[64:128, pr, ts(i, 128)], QT[64:128, pr, qs],
                    start=True, stop=True, tile_position=(64, 0),
                )
                pt = ppool.tile([128, 2, 512], bf16, tag="pt")
                nc.scalar.activation(
                    out=pt[:, :, 0:ncol], in_=sps[:, :, 0:ncol],
                    func=Exp, scale=SCALE, bias=mb_sb[:, i : i + 1],
                )
                if 128 * i >= 512 * ch:  # diagonal block: causal mask
                    nc.vector.tensor_mul(
                        out=pt[:, 0, 0:128], in0=pt[:, 0, 0:128], in1=tri_b
                    )
                    nc.vector.tensor_mul(
                        out=pt[:, 1, 0:128], in0=pt[:, 1, 0:128], in1=tri_b
                    )
                v3 = VS[:, i, :].rearrange("p (h j) -> p h j", j=65)
                nc.tensor.matmul(
                    ypA[:, off : off + ncol], v3[:, 2 * pr, :], pt[:, 0, 0:ncol],
                    start=(i == 0), stop=(i == nsb - 1),
                )
                nc.tensor.matmul(
                    ypB[:, off : off + ncol], v3[:, 2 * pr + 1, :], pt[:, 1, 0:ncol],
                    start=(i == 0), stop=(i == nsb - 1),
                )
            # normalize: row 64 of yp holds the softmax denominator.
            # Reciprocal + copy free the PSUM banks fast; the broadcast and
            # multiply run off the critical path on SBUF copies.
            if ch == NCH - 1:
                # tail chunk: broadcast 1/l on the (now idle) PE via an
                # outer product instead of the DRAM bounce — shorter chain
                yraw3 = rpool.tile([128, 512], f32, tag="yraw3")
                for hh, yp in ((0, ypA), (1, ypB)):
                    r1 = rpool.tile([1, 512], bf16, tag=f"r1{hh}")
                    with nc.allow_low_precision(reason="softmax 1/l in bf16"):
                        nc.vector.reciprocal(out=r1, in_=yp[64:65, :])
                    nc.vector.tensor_copy(
                        out=yraw3[64 * hh : 64 * hh + 64, :], in_=yp[0:64, :]
                    )
                    rb_ps = psum1.tile([128, 512], f32, tag="p1")
                    nc.tensor.matmul(rb_ps, ones_r, r1, start=True, stop=True)
                    nc.vector.tensor_mul(
                        out=YT[64 * hh : 64 * hh + 64, pr, ts(ch, 512)],
                        in0=yraw3[64 * hh : 64 * hh + 64, :],
                        in1=rb_ps[64 * hh : 64 * hh + 64, :],
                    )
                return
            yraw = rpool.tile([128, 512], bf16, tag="yraw")
            idx = pr * NCH + ch
            rDi = rD[2 * idx : 2 * idx + 2, :]
            for hh, yp in ((0, ypA), (1, ypB)):
                r1 = rpool.tile([1, 512], bf16, tag=f"r1{hh}")
                with nc.allow_low_precision(reason="softmax 1/l in bf16"):
                    nc.vector.reciprocal(out=r1, in_=yp[64:65, :])
                nc.sync.dma_start(out=rDi[hh : hh + 1, :], in_=r1)
                nc.vector.tensor_copy(
                    out=yraw[64 * hh : 64 * hh + 64, :], in_=yp[0:64, :]
                )
            # broadcast both rows to all 128 partitions via a DRAM bounce
            # with a step-0 partition AP (SBUF APs can't have zero p-step);
            # full-height so each TT reads in0/in1 at the same base partition
            rb = rpool.tile([128, 2, 512], bf16, tag="rb")
            bc = bass.AP(
                tensor=rDi.tensor, offset=rDi.offset,
                ap=[[0, 128], [512, 2], [1, 512]],
            )
            nc.sync.dma_start(out=rb, in_=bc)
            for hh in (0, 1):
                nc.vector.tensor_mul(
                    out=YT[64 * hh : 64 * hh + 64, pr, ts(ch, 512)],
                    in0=yraw[64 * hh : 64 * hh + 64, :],
                    in1=rb[64 * hh : 64 * hh + 64, hh, :],
                )

        def emit_proj(ch):
            # out^T[:, chunk ch] only needs Y[:, :, ch] — run as soon as all
            # pairs' attention for chunk ch is done.  Contract k starting
            # from the last-finishing pair so a chain never parks mid-way on
            # a psum slot waiting for pair 3's Y.
            korder = [NM - 1] + list(range(NM - 1))
            for m in range(C // 128):
                pp = psum1.tile([128, 512], f32, tag="p1")
                for j, k in enumerate(korder):
                    nc.tensor.matmul(
                        pp, WP[:, k, ts(m, 128)], YT[:, k, ts(ch, 512)],
                        start=(j == 0), stop=(j == NM - 1),
                    )
                osb = opool.tile([128, 512], f32, tag="o")
                nc.vector.tensor_scalar(
                    out=osb, in0=pp,
                    scalar1=bp_sb[:, m : m + 1], scalar2=None, op0=ADD,
                )
                eng = nc.sync if ch == NCH - 1 else nc.gpsimd
                eng.dma_start(out=out_d[ts(m, 128), ts(ch, 512)], in_=osb)

        # ------- QKV, attention and proj interleaved per chunk -------
        # attention(pr, ch) needs Q/K chunks 0..ch and V s-blocks 0..4ch+3,
        # all available once QKV chunk ch is emitted.  proj(ch-1) is slotted
        # after QK(ch) so its psum1 chains never interleave with QKV's, and
        # its PE work fills the ACT-bound attention windows.
        for ch in range(NCH):
            if ch == 0:
                # chunk 0: V first, then attention right behind each pair's
                # QK so the exps (ACT) start as early as possible
                for t in range(4):
                    emit_v(t)
                for pr in range(NM):
                    emit_qk(pr, 0)
                    emit_attention(pr, 0)
                continue
            for m in range(NM):
                emit_qk(m, ch)
            for t in range(4 * ch, 4 * ch + 4):
                emit_v(t)
            if ch == 1:  # W_proj loads, needed just before proj(0)
                for k in range(NM):
                    nc.gpsimd.dma_start(out=WP[:, k, :], in_=wp_d[ts(k, 128), :])
            emit_proj(ch - 1)
            for pr in range(NM):
                emit_attention(pr, ch)
        emit_proj(NCH - 1)

    if not nc.is_finalized():
        nc.finalize()
    return nc


def make_in_maps(x, attn_mask, W_qkv, b_qkv, W_proj, b_proj):
    """Shard full inputs into 8 per-core input maps (bf16 matmul operands)."""
    import ml_dtypes

    bf16 = ml_dtypes.bfloat16
    x = np.asarray(x, dtype=np.float32)
    attn_mask = np.asarray(attn_mask)
    W_qkv = np.asarray(W_qkv, dtype=np.float32)
    b_qkv = np.asarray(b_qkv, dtype=np.float32)
    W_proj = np.asarray(W_proj, dtype=np.float32)
    b_proj = np.asarray(b_proj, dtype=np.float32)

    in_maps = []
    for c in range(8):
        b, g = c // 2, c % 2
        s = 512 * g
        wq = W_qkv[:, s : s + 512]
        wk = W_qkv[:, C + s : C + s + 512]
        wv = W_qkv[:, 2 * C + s : 2 * C + s + 512]
        bv = b_qkv[2 * C + s : 2 * C + s + 512]
        bv520 = np.zeros(8 * 65, dtype=np.float32)
        bv520.reshape(8, 65)[:, :64] = bv.reshape(8, 64)
        mb = np.where(
            attn_mask[b].reshape(NT, 128).T.astype(np.int64) != 0, 0.0, NEG
        ).astype(np.float32)
        in_maps.append({
            "xt": np.ascontiguousarray(x[b].T).astype(bf16),
            "wqkv": np.ascontiguousarray(
                np.concatenate([wq, wk, wv], axis=1)
            ).astype(bf16),
            "bq": np.ascontiguousarray(b_qkv[s : s + 512]),
            "bk": np.ascontiguousarray(b_qkv[C + s : C + s + 512]),
            "bv520": bv520.astype(bf16),
            "wproj": np.ascontiguousarray(W_proj[s : s + 512, :]).astype(bf16),
            "bproj": (b_proj if g == 0 else np.zeros(C, dtype=np.float32)).copy(),
            "maskbias": np.ascontiguousarray(mb),
            "tri": np.triu(np.ones((128, 128), dtype=np.float32)).astype(bf16),
        })
    return in_maps


def unshard(results):
    """results: list of 8 dicts with 'out' (C, T) partial transposed outputs."""
    outs = []
    for b in range(B):
        part = results[2 * b]["out"] + results[2 * b + 1]["out"]
        outs.append(part.T)
    return np.ascontiguousarray(np.stack(outs)).astype(np.float32)


def kernel(x, attn_mask, W_qkv, b_qkv, W_proj, b_proj):
    from concourse.bass_utils import run_bass_kernel_spmd

    nc = build_nc()
    in_maps = make_in_maps(x, attn_mask, W_qkv, b_qkv, W_proj, b_proj)
    res = run_bass_kernel_spmd(nc, in_maps, core_ids=list(range(8)))
    kernel.last_results = res
    return unshard([r for r in res.results])


# revision 52
# speedup vs baseline: 1.0353x; 1.0294x over previous
"""Distributed causal self-attention for TRN2 (8 NeuronCores).

Problem: B=4, T=2048, C=1024, H=16 heads, D=64.
  qkv = x @ W_qkv + b_qkv ; causal softmax attention ; y @ W_proj + b_proj

Sharding (8 cores): core c -> batch b = c//2, head-group g = c%2
(heads 8g..8g+7).  Each core computes, for its (b, g):
  Q^T/K^T (hd, T) and V (T, hd) for its 8 heads (hd = 512),
  flash-style causal attention in S^T = K @ Q^T layout (s on partitions,
  head pairs row/col-packed on the PE array),
  partial out^T = (Y @ W_proj[rows g])^T  (1024, 2048).
Host unshard: out[b] = (part[2b] + part[2b+1]).T  (b_proj added on-device
by the g==0 core only).

Host-side shard prep casts matmul operands to bf16 and pre-transposes x
(to x^T, the layout every projection consumes); PSUM accumulates fp32.
Softmax runs without max-subtraction (scores ~ N(0,1) here; exp is safe
in fp32), the padding mask is folded into the exp bias, and the causal
mask is a (128,128) triangular multiply on diagonal tiles.  The softmax
denominator comes free from an all-ones 65th column appended to V;
normalization happens on the (64, 512) attention output tiles via a
DRAM-bounced partition broadcast of 1/l (PE outer-product broadcast on
the tail chunk, where the PE is otherwise idle).
"""

from contextlib import ExitStack

import numpy as np

# ---------------- constants (hardcoded per problem spec) ----------------
B, T, C, H, D = 4, 2048, 1024, 16, 64
HD = 512          # heads-per-core * D = 8 * 64
NK = C // 128     # 8 contraction tiles over C
NM = HD // 128    # 4 tiles over the per-core head dim (also = head pairs)
NT = T // 128     # 16 s/T blocks
NCH = T // 512    # 4 q-chunks
SCALE = 1.0 / np.sqrt(D)  # 0.125
NEG = -30.0       # "minus infinity" for the padding mask bias


def build_nc():
    import concourse.bass as bass
    import concourse.mybir as mybir
    import concourse.tile as tile
    from concourse.bacc import Bacc

    f32 = mybir.dt.float32
    bf16 = mybir.dt.bfloat16
    Exp = mybir.ActivationFunctionType.Exp
    ADD = mybir.AluOpType.add

    nc = Bacc()

    xt_d = nc.dram_tensor("xt", (C, T), bf16, kind="ExternalInput")
    wqkv_d = nc.dram_tensor("wqkv", (C, 3 * HD), bf16, kind="ExternalInput")
    bq_d = nc.dram_tensor("bq", (HD,), f32, kind="ExternalInput")
    bk_d = nc.dram_tensor("bk", (HD,), f32, kind="ExternalInput")
    bv_d = nc.dram_tensor("bv520", (8 * 65,), bf16, kind="ExternalInput")
    wp_d = nc.dram_tensor("wproj", (HD, C), bf16, kind="ExternalInput")
    bp_d = nc.dram_tensor("bproj", (C,), f32, kind="ExternalInput")
    mb_d = nc.dram_tensor("maskbias", (128, NT), f32, kind="ExternalInput")
    tri_d = nc.dram_tensor("tri", (128, 128), bf16, kind="ExternalInput")
    out_d = nc.dram_tensor("out", (C, T), f32, kind="ExternalOutput")

    ts = bass.ts

    with ExitStack() as ctx:
        tc = ctx.enter_context(tile.TileContext(nc))
        persist = ctx.enter_context(tc.tile_pool(name="persist", bufs=1))
        small = ctx.enter_context(tc.tile_pool(name="small", bufs=1))
        ppool = ctx.enter_context(tc.tile_pool(name="ppool", bufs=6))
        rpool = ctx.enter_context(tc.tile_pool(name="rpool", bufs=3))
        opool = ctx.enter_context(tc.tile_pool(name="opool", bufs=3))
        dram = ctx.enter_context(tc.tile_pool(name="dram", bufs=1, space="DRAM"))
        psum1 = ctx.enter_context(tc.tile_pool(name="psum1", bufs=2, space="PSUM"))
        psS = ctx.enter_context(tc.tile_pool(name="psS", bufs=2, space="PSUM"))
        psY = ctx.enter_context(tc.tile_pool(name="psY", bufs=2, space="PSUM"))

        # ---------------- persistent SBUF tensors ----------------
        XT = persist.tile([128, NK, T], bf16, tag="xt")        # x^T  (C, T)
        WQKV = persist.tile([128, NK, 3 * HD], bf16, tag="wqkv")
        WP = persist.tile([128, NM, C], bf16, tag="wp")
        QT = persist.tile([128, NM, T], bf16, tag="qt")        # Q^T (hd, T)
        KT = persist.tile([128, NM, T], bf16, tag="kt")
        VS = persist.tile([128, NT, 8 * 65], bf16, tag="vs")   # V+ones per s-block
        YT = persist.tile([128, NM, T], bf16, tag="yt")        # normalized Y^T

        rD = dram.tile([2 * NM * NCH, 512], bf16, tag="rd")    # 1/l bounce

        # small constants
        bq_sb = small.tile([128, NM], f32, tag="bq")
        bk_sb = small.tile([128, NM], f32, tag="bk")
        bp_sb = small.tile([128, C // 128], f32, tag="bp")
        mb_sb = small.tile([128, NT], f32, tag="mb")
        tri_b = small.tile([128, 128], bf16, tag="trib")
        bvb = small.tile([128, 8 * 65], bf16, tag="bvb")
        ones_r = small.tile([1, 128], bf16, tag="ones_r")
        nc.vector.memset(ones_r, 1.0)

        nc.gpsimd.dma_start(out=bq_sb, in_=bq_d.rearrange("(m p) -> p m", p=128))
        nc.gpsimd.dma_start(out=bk_sb, in_=bk_d.rearrange("(m p) -> p m", p=128))
        nc.gpsimd.dma_start(out=bp_sb, in_=bp_d.rearrange("(m p) -> p m", p=128))
        nc.gpsimd.dma_start(out=mb_sb, in_=mb_d[:, :])
        nc.gpsimd.dma_start(out=tri_b, in_=tri_d[:, :])
        # broadcast the (520,) v-bias row to 128 partitions via a step-0 AP
        bvb_bcast = bass.AP(tensor=bv_d, offset=0, ap=[[0, 128], [1, 8 * 65]])
        nc.gpsimd.dma_start(out=bvb, in_=bvb_bcast)

        # ---------------- input loads (bf16, no staging) ----------------
        hwdge = [nc.sync, nc.scalar]
        for k in range(NK):
            hwdge[k % 2].dma_start(out=XT[:, k, :], in_=xt_d[ts(k, 128), :])
        for k in range(NK):
            nc.gpsimd.dma_start(out=WQKV[:, k, :], in_=wqkv_d[ts(k, 128), :])

        Ident = mybir.ActivationFunctionType.Identity

        def emit_qk(m, ch):
            # early chunks: drain psum->SBUF on the ACT engine (idle during
            # the QKV-heavy window) so the DVE queue never gates the first
            # S-matmuls' K^T/Q^T readiness
            on_act = ch < 2

            psq = psum1.tile([128, 512], f32, tag="p1")
            for k in range(NK):
                nc.tensor.matmul(
                    psq, WQKV[:, k, ts(m, 128)], XT[:, k, ts(ch, 512)],
                    start=(k == 0), stop=(k == NK - 1),
                )
            if on_act:
                nc.scalar.activation(
                    out=QT[:, m, ts(ch, 512)], in_=psq, func=Ident,
                    bias=bq_sb[:, m : m + 1],
                )
            else:
                nc.vector.tensor_scalar(
                    out=QT[:, m, ts(ch, 512)], in0=psq,
                    scalar1=bq_sb[:, m : m + 1], scalar2=None, op0=ADD,
                )
            psk = psum1.tile([128, 512], f32, tag="p1")
            for k in range(NK):
                nc.tensor.matmul(
                    psk, WQKV[:, k, HD + 128 * m : HD + 128 * (m + 1)],
                    XT[:, k, ts(ch, 512)],
                    start=(k == 0), stop=(k == NK - 1),
                )
            if on_act:
                nc.scalar.activation(
                    out=KT[:, m, ts(ch, 512)], in_=psk, func=Ident,
                    bias=bk_sb[:, m : m + 1],
                )
            else:
                nc.vector.tensor_scalar(
                    out=KT[:, m, ts(ch, 512)], in0=psk,
                    scalar1=bk_sb[:, m : m + 1], scalar2=None, op0=ADD,
                )

        def emit_v(t):
            psv = psum1.tile([128, 512], f32, tag="p1")
            for k in range(NK):
                nc.tensor.matmul(
                    psv, XT[:, k, ts(t, 128)], WQKV[:, k, 2 * HD : 3 * HD],
                    start=(k == 0), stop=(k == NK - 1),
                )
            v3 = VS[:, t, :].rearrange("p (h j) -> p h j", j=65)
            nc.vector.memset(v3[:, :, 64:65], 1.0)
            if t < 8:  # early chunks: drain on the then-idle ACT engine
                nc.scalar.activation(
                    out=v3[:, :, 0:64],
                    in_=psv.rearrange("p (h j) -> p h j", j=64), func=Ident,
                )
            else:
                nc.vector.tensor_copy(
                    out=v3[:, :, 0:64], in_=psv.rearrange("p (h j) -> p h j", j=64)
                )
            nc.vector.tensor_add(out=VS[:, t, :], in0=VS[:, t, :], in1=bvb)

        def emit_attention(pr, ch):
            ypA = psY.tile([65, 512], f32, tag="yp")
            ypB = psY.tile([65, 512], f32, tag="yp")
            nsb = 4 * ch + 4
            for i in range(nsb):
                off = max(0, 128 * i - 512 * ch)
                ncol = 512 - off
                qs = slice(512 * ch + off, 512 * (ch + 1))
                sps = psS.tile([128, 2, 512], f32, tag="s")
                nc.tensor.matmul(
                    sps[:, 0, 0:ncol], KT[0:64, pr, ts(i, 128)], QT[0:64, pr, qs],
                    start=True, stop=True, tile_position=(0, 0),
                )
                nc.tensor.matmul(
                    sps[:, 1, 0:ncol], KT[64:128, pr, ts(i, 128)], QT[64:128, pr, qs],
                    start=True, stop=True, tile_position=(64, 0),
                )
                pt = ppool.tile([128, 2, 512], bf16, tag="pt")
                nc.scalar.activation(
                    out=pt[:, :, 0:ncol], in_=sps[:, :, 0:ncol],
                    func=Exp, scale=SCALE, bias=mb_sb[:, i : i + 1],
                )
                if 128 * i >= 512 * ch:  # diagonal block: causal mask
                    nc.vector.tensor_mul(
                        out=pt[:, 0, 0:128], in0=pt[:, 0, 0:128], in1=tri_b
                    )
                    nc.vector.tensor_mul(
                        out=pt[:, 1, 0:128], in0=pt[:, 1, 0:128], in1=tri_b
                    )
                v3 = VS[:, i, :].rearrange("p (h j) -> p h j", j=65)
                nc.tensor.matmul(
                    ypA[:, off : off + ncol], v3[:, 2 * pr, :], pt[:, 0, 0:ncol],
                    start=(i == 0), stop=(i == nsb - 1),
                )
                nc.tensor.matmul(
                    ypB[:, off : off + ncol], v3[:, 2 * pr + 1, :], pt[:, 1, 0:ncol],
                    start=(i == 0), stop=(i == nsb - 1),
                )
            # normalize: row 64 of yp holds the softmax denominator.
            # Reciprocal + copy free the PSUM banks fast; the broadcast and
            # multiply run off the critical path on SBUF copies.
            if ch == NCH - 1:
                # tail chunk: broadcast 1/l on the (now idle) PE via an
                # outer product instead of the DRAM bounce — shorter chain
                yraw3 = rpool.tile([128, 512], f32, tag="yraw3")
                for hh, yp in ((0, ypA), (1, ypB)):
                    r1 = rpool.tile([1, 512], bf16, tag=f"r1{hh}")
                    with nc.allow_low_precision(reason="softmax 1/l in bf16"):
                        nc.vector.reciprocal(out=r1, in_=yp[64:65, :])
                    nc.vector.tensor_copy(
                        out=yraw3[64 * hh : 64 * hh + 64, :], in_=yp[0:64, :]
                    )
                    rb_ps = psum1.tile([128, 512], f32, tag="p1")
                    nc.tensor.matmul(rb_ps, ones_r, r1, start=True, stop=True)
                    nc.vector.tensor_mul(
                        out=YT[64 * hh : 64 * hh + 64, pr, ts(ch, 512)],
                        in0=yraw3[64 * hh : 64 * hh + 64, :],
                        in1=rb_ps[64 * hh : 64 * hh + 64, :],
                    )
                return
            yraw = rpool.tile([128, 512], bf16, tag="yraw")
            idx = pr * NCH + ch
            rDi = rD[2 * idx : 2 * idx + 2, :]
            for hh, yp in ((0, ypA), (1, ypB)):
                r1 = rpool.tile([1, 512], bf16, tag=f"r1{hh}")
                with nc.allow_low_precision(reason="softmax 1/l in bf16"):
                    nc.vector.reciprocal(out=r1, in_=yp[64:65, :])
                nc.sync.dma_start(out=rDi[hh : hh + 1, :], in_=r1)
                nc.vector.tensor_copy(
                    out=yraw[64 * hh : 64 * hh + 64, :], in_=yp[0:64, :]
                )
            # broadcast both rows to all 128 partitions via a DRAM bounce
            # with a step-0 partition AP (SBUF APs can't have zero p-step);
            # full-height so each TT reads in0/in1 at the same base partition
            rb = rpool.tile([128, 2, 512], bf16, tag="rb")
            bc = bass.AP(
                tensor=rDi.tensor, offset=rDi.offset,
                ap=[[0, 128], [512, 2], [1, 512]],
            )
            nc.sync.dma_start(out=rb, in_=bc)
            for hh in (0, 1):
                nc.vector.tensor_mul(
                    out=YT[64 * hh : 64 * hh + 64, pr, ts(ch, 512)],
                    in0=yraw[64 * hh : 64 * hh + 64, :],
                    in1=rb[64 * hh : 64 * hh + 64, hh, :],
                )

        def emit_proj(ch):
            # out^T[:, chunk ch] only needs Y[:, :, ch] — run as soon as all
            # pairs' attention for chunk ch is done.  Contract k starting
            # from the last-finishing pair so a chain never parks mid-way on
            # a psum slot waiting for pair 3's Y.
            korder = [NM - 1] + list(range(NM - 1))
            for m in range(C // 128):
                pp = psum1.tile([128, 512], f32, tag="p1")
                for j, k in enumerate(korder):
                    nc.tensor.matmul(
                        pp, WP[:, k, ts(m, 128)], YT[:, k, ts(ch, 512)],
                        start=(j == 0), stop=(j == NM - 1),
                    )
                osb = opool.tile([128, 512], f32, tag="o")
                nc.vector.tensor_scalar(
                    out=osb, in0=pp,
                    scalar1=bp_sb[:, m : m + 1], scalar2=None, op0=ADD,
                )
                eng = nc.sync if ch == NCH - 1 else nc.gpsimd
                eng.dma_start(out=out_d[ts(m, 128), ts(ch, 512)], in_=osb)

        # ------- QKV, attention and proj interleaved per chunk -------
        # attention(pr, ch) needs Q/K chunks 0..ch and V s-blocks 0..4ch+3,
        # all available once QKV chunk ch is emitted.  proj(ch-1) is slotted
        # after QK(ch) so its psum1 chains never interleave with QKV's, and
        # its PE work fills the ACT-bound attention windows.
        for ch in range(NCH):
            if ch == 0:
                # chunk 0: V first, then attention right behind each pair's
                # QK so the exps (ACT) start as early as possible
                for t in range(4):
                    emit_v(t)
                for pr in range(NM):
                    emit_qk(pr, 0)
                    emit_attention(pr, 0)
                continue
            for m in range(NM):
                emit_qk(m, ch)
            for t in range(4 * ch, 4 * ch + 4):
                emit_v(t)
            if ch == 1:  # W_proj loads, needed just before proj(0)
                for k in range(NM):
                    nc.gpsimd.dma_start(out=WP[:, k, :], in_=wp_d[ts(k, 128), :])
            emit_proj(ch - 1)
            for pr in range(NM):
                emit_attention(pr, ch)
        emit_proj(NCH - 1)

    if not nc.is_finalized():
        nc.finalize()
    return nc


def make_in_maps(x, attn_mask, W_qkv, b_qkv, W_proj, b_proj):
    """Shard full inputs into 8 per-core input maps (bf16 matmul operands)."""
    import ml_dtypes

    bf16 = ml_dtypes.bfloat16
    x = np.asarray(x, dtype=np.float32)
    attn_mask = np.asarray(attn_mask)
    W_qkv = np.asarray(W_qkv, dtype=np.float32)
    b_qkv = np.asarray(b_qkv, dtype=np.float32)
    W_proj = np.asarray(W_proj, dtype=np.float32)
    b_proj = np.asarray(b_proj, dtype=np.float32)

    in_maps = []
    for c in range(8):
        b, g = c // 2, c % 2
        s = 512 * g
        wq = W_qkv[:, s : s + 512]
        wk = W_qkv[:, C + s : C + s + 512]
        wv = W_qkv[:, 2 * C + s : 2 * C + s + 512]
        bv = b_qkv[2 * C + s : 2 * C + s + 512]
        bv520 = np.zeros(8 * 65, dtype=np.float32)
        bv520.reshape(8, 65)[:, :64] = bv.reshape(8, 64)
        mb = np.where(
            attn_mask[b].reshape(NT, 128).T.astype(np.int64) != 0, 0.0, NEG
        ).astype(np.float32)
        in_maps.append({
            "xt": np.ascontiguousarray(x[b].T).astype(bf16),
            "wqkv": np.ascontiguousarray(
                np.concatenate([wq, wk, wv], axis=1)
            ).astype(bf16),
            "bq": np.ascontiguousarray(b_qkv[s : s + 512]),
            "bk": np.ascontiguousarray(b_qkv[C + s : C + s + 512]),
            "bv520": bv520.astype(bf16),
            "wproj": np.ascontiguousarray(W_proj[s : s + 512, :]).astype(bf16),
            "bproj": (b_proj if g == 0 else np.zeros(C, dtype=np.float32)).copy(),
            "maskbias": np.ascontiguousarray(mb),
            "tri": np.triu(np.ones((128, 128), dtype=np.float32)).astype(bf16),
        })
    return in_maps


def unshard(results):
    """results: list of 8 dicts with 'out' (C, T) partial transposed outputs."""
    outs = []
    for b in range(B):
        part = results[2 * b]["out"] + results[2 * b + 1]["out"]
        outs.append(part.T)
    return np.ascontiguousarray(np.stack(outs)).astype(np.float32)


def kernel(x, attn_mask, W_qkv, b_qkv, W_proj, b_proj):
    from concourse.bass_utils import run_bass_kernel_spmd

    nc = build_nc()
    in_maps = make_in_maps(x, attn_mask, W_qkv, b_qkv, W_proj, b_proj)
    res = run_bass_kernel_spmd(nc, in_maps, core_ids=list(range(8)))
    kernel.last_results = res
    return unshard([r for r in res.results])
